# revision 6
# baseline (speedup 1.0000x reference)
"""Trainium2 Bass kernel for a Bahdanau-style batch attention layer.

  A = rnn @ W1.T            [S, D]    (W1 = W_lin[:, :DU])
  B = tgt @ W2.T + b_lin    [T, D]    (W2 = W_lin[:, DU:])
  scores[t, s] = w_score . tanh(A[s] + B[t])   (+ b_score, softmax-invariant)
  out = softmax_s(scores) @ rnn                [T, DU]

Sharding: T (target rows) split across 8 NeuronCores; rnn/W replicated.
Host staging pre-transposes/casts the replicated operands so each core's
layout is [d-on-partitions, *-on-free]: the outer-sum A[s]+B[t] is a
per-partition tensor_scalar add on DVE, tanh is a long-free-dim ScalarE
activation (the roofline engine: T*S*D/8 = 33.5M LUT evals per core), and
the score reduction over d is 8 accumulating M=1 matmuls per target row.
"""

import sys
import types

import numpy as np

S = 512
T = 512
DU = 512
DT = 512
D = DU + DT
NCORES = 8
TL = T // NCORES  # 64 target rows per core
TB = 8            # t-block size in the main loop
NTB = TL // TB    # 8 blocks
KD = D // 128     # 8 tiles over d
KS = S // 128     # 4 tiles over s


def _ensure_concourse():
    try:
        import concourse  # noqa: F401
    except ImportError:
        for p in ("/opt/trn_rl_repo", "/root/.axon_site/_ro/trn_rl_repo"):
            if p not in sys.path:
                sys.path.append(p)


def _wire_ntff_hook():
    """Register the NTFF profile hook if the image's antenv lacks it."""
    try:
        import antenv
        if hasattr(antenv, "axon_hooks"):
            return
        mod = types.ModuleType("antenv.axon_hooks")
        mod._hook = None
        def set_axon_ntff_profile_hook(h):
            mod._hook = h
        def get_axon_ntff_profile_hook():
            return mod._hook
        mod.set_axon_ntff_profile_hook = set_axon_ntff_profile_hook
        mod.get_axon_ntff_profile_hook = get_axon_ntff_profile_hook
        sys.modules["antenv.axon_hooks"] = mod
        antenv.axon_hooks = mod
        from trn_agent_boot.trn_boot import _ntff_profile_via_ctypes
        hook = _ntff_profile_via_ctypes("/opt/axon/libaxon_pjrt.so")
        if hook is not None:
            set_axon_ntff_profile_hook(hook)
    except Exception:
        pass


_NC_CACHE = {}


def build_program():
    if "nc" in _NC_CACHE:
        return _NC_CACHE["nc"]
    _ensure_concourse()
    import concourse.bacc as bacc
    import concourse.tile as tile
    from concourse import mybir
    from concourse.masks import make_identity

    f32 = mybir.dt.float32
    bf16 = mybir.dt.bfloat16
    AF = mybir.ActivationFunctionType
    ALU = mybir.AluOpType
    AX = mybir.AxisListType

    nc = bacc.Bacc("TRN2", target_bir_lowering=False, debug=False)

    rnnb_d = nc.dram_tensor("rnnb", [S, DU], bf16, kind="ExternalInput")
    rnnt_d = nc.dram_tensor("rnnt", [DU, S], bf16, kind="ExternalInput")
    tgtt_d = nc.dram_tensor("tgtt", [DT, TL], bf16, kind="ExternalInput")
    wlint_d = nc.dram_tensor("wlint", [D, D], bf16, kind="ExternalInput")
    blin_d = nc.dram_tensor("blin", [128, KD], f32, kind="ExternalInput")
    wsb_d = nc.dram_tensor("wsb", [128, KD], bf16, kind="ExternalInput")
    out_d = nc.dram_tensor("out", [TL, DU], f32, kind="ExternalOutput")

    with tile.TileContext(nc) as tc:
        with (
            tc.tile_pool(name="consts", bufs=1) as consts,
            tc.tile_pool(name="misc", bufs=4) as misc,
            tc.tile_pool(name="hbuf", bufs=2) as hbuf,
            tc.tile_pool(name="ps", bufs=8, space="PSUM") as psp,
        ):
            # ---------------- constants / inputs ----------------
            junk = consts.tile([128, 1], f32)
            nc.gpsimd.memset(junk[:], 0.0)
            # dummy activation: pulls the ~2.7us ACT table load to t=0
            nc.scalar.activation(junk[:], junk[:], AF.Tanh)
            ident_bf = consts.tile([128, 128], bf16)
            make_identity(nc, ident_bf)

            rnnT = consts.tile([128, KS, S], bf16)     # [p(k), ki, s]
            for ki in range(KS):
                nc.sync.dma_start(rnnT[:, ki, :], rnnt_d[ki * 128:(ki + 1) * 128, :])
            tgtT = consts.tile([128, KS, TL], bf16)    # [p(k), ki, t]
            for ki in range(KS):
                nc.sync.dma_start(tgtT[:, ki, :], tgtt_d[ki * 128:(ki + 1) * 128, :])
            bl_sb = consts.tile([128, KD], f32)
            nc.sync.dma_start(bl_sb[:], blin_d[:])
            ws_bf = consts.tile([128, KD], bf16)
            nc.sync.dma_start(ws_bf[:], wsb_d[:])
            # W^T loaded block-wise, in the order the per-dj prologue consumes it
            wlT = consts.tile([128, KD, KD, 128], bf16)  # [p(k), ki, dj, 128(d)]

            AT_sb = consts.tile([128, KD, S], bf16)    # [p(d), dj, s]
            BbT_sb = consts.tile([128, KD, TL], f32)   # [p(d), dj, t]
            scores_sb = consts.tile([TL, S], f32)
            h_tiles = [None] * NTB

            def emit_h(tb, dj):
                h = h_tiles[tb]
                for tl in range(TB):
                    t = tb * TB + tl
                    nc.vector.tensor_scalar_add(
                        h[:, dj, tl * S:(tl + 1) * S],
                        AT_sb[:, dj, :],
                        BbT_sb[:, dj, t:t + 1],
                    )
                nc.scalar.activation(h[:, dj, :], h[:, dj, :], AF.Tanh)

            # ---------------- prologue: A^T, Bb^T (+ block-0 h) ----------------
            h_tiles[0] = hbuf.tile([128, KD, TB * S], bf16, tag="h", name="h0")
            for dj in range(KD):
                for ki in range(KD):
                    nc.sync.dma_start(
                        wlT[:, ki, dj, :],
                        wlint_d[ki * 128:(ki + 1) * 128, dj * 128:(dj + 1) * 128],
                    )
                at_ps = psp.tile([128, 512], f32, tag="ps")
                for ki in range(KS):
                    nc.tensor.matmul(
                        at_ps[:], wlT[:, ki, dj, :], rnnT[:, ki, :],
                        start=(ki == 0), stop=(ki == KS - 1),
                    )
                nc.vector.tensor_copy(AT_sb[:, dj, :], at_ps[:])
                bt_ps = psp.tile([128, 512], f32, tag="ps")
                for ki in range(KS):
                    nc.tensor.matmul(
                        bt_ps[:, :TL], wlT[:, KS + ki, dj, :],
                        tgtT[:, ki, :],
                        start=(ki == 0), stop=(ki == KS - 1),
                    )
                nc.vector.tensor_scalar_add(
                    BbT_sb[:, dj, :], bt_ps[:, :TL], bl_sb[:, dj:dj + 1]
                )
                emit_h(0, dj)
            # tail-only operand: loaded after the prologue-critical DMAs
            rnn_bf = consts.tile([128, KS, DU], bf16)  # [p(s), si, du]
            for si in range(KS):
                nc.sync.dma_start(rnn_bf[:, si, :], rnnb_d[si * 128:(si + 1) * 128, :])

            # ---------------- main loop ----------------
            def emit_block(tb):
                h_tiles[tb] = hbuf.tile([128, KD, TB * S], bf16, tag="h", name=f"h{tb}")
                for dj in range(KD):
                    emit_h(tb, dj)

            def emit_scores(tb):
                h = h_tiles[tb]
                scs = [psp.tile([1, 512], f32, tag="ps", name=f"sc{tb}_{i}") for i in range(TB)]
                for dj in range(KD):
                    for tl in range(TB):
                        nc.tensor.matmul(
                            scs[tl][:], ws_bf[:, dj:dj + 1],
                            h[:, dj, tl * S:(tl + 1) * S],
                            start=(dj == 0), stop=(dj == KD - 1),
                        )
                for tl in range(TB):
                    t = tb * TB + tl
                    stage = misc.tile([1, 512], f32, tag="stage")
                    nc.vector.tensor_copy(stage[:], scs[tl][:])
                    nc.sync.dma_start(scores_sb[t:t + 1, :], stage[:])

            for tb in range(1, NTB):
                emit_block(tb)
                emit_scores(tb - 1)
            emit_scores(NTB - 1)

            # ---------------- softmax over s ----------------
            # |scores| <= sum|w_d| (tanh in [-1,1]) is far below exp overflow;
            # skip the max-subtraction (softmax is shift-invariant)
            e_sb = misc.tile([TL, S], f32, tag="esb")
            nc.scalar.activation(e_sb[:], scores_sb[:], AF.Exp)
            ssum = misc.tile([TL, 1], f32, tag="ssum")
            nc.vector.tensor_reduce(ssum[:], e_sb[:], axis=AX.X, op=ALU.add)
            rsum = misc.tile([TL, 1], f32, tag="rsum")
            nc.vector.reciprocal(rsum[:], ssum[:])
            wts = misc.tile([TL, S], bf16, tag="wts")
            nc.vector.tensor_scalar_mul(wts[:], e_sb[:], rsum[:, 0:1])

            # ---------------- out = wts @ rnn ----------------
            wsT = misc.tile([128, KS, TL], bf16, tag="wsT")
            for sj in range(KS):
                tp = psp.tile([128, 512], bf16, tag="ps")
                nc.tensor.transpose(
                    tp[:128, :TL], wts[:, sj * 128:(sj + 1) * 128],
                    ident_bf[:TL, :TL],
                )
                nc.vector.tensor_copy(wsT[:, sj, :], tp[:, :TL])
            out_ps = psp.tile([TL, DU], f32, tag="ps")
            for ki in range(KS):
                nc.tensor.matmul(
                    out_ps[:], wsT[:, ki, :], rnn_bf[:, ki, :],
                    start=(ki == 0), stop=(ki == KS - 1),
                )
            out_sb = misc.tile([TL, DU], f32, tag="osb")
            nc.vector.tensor_copy(out_sb[:], out_ps[:])
            nc.sync.dma_start(out_d[:], out_sb[:])

    nc.compile()
    _NC_CACHE["nc"] = nc
    return nc


def make_in_maps(rnn_outputs, target, W_lin, b_lin, w_score):
    import ml_dtypes
    bf = ml_dtypes.bfloat16
    rnn = np.asarray(rnn_outputs, dtype=np.float32)
    tgt = np.asarray(target, dtype=np.float32)
    wlin = np.asarray(W_lin, dtype=np.float32)
    blin = np.asarray(b_lin, dtype=np.float32).reshape(KD, 128).T.copy()
    wsb = np.asarray(w_score, dtype=np.float32).reshape(KD, 128).T.astype(bf)
    rnnb = rnn.astype(bf)
    rnnt = np.ascontiguousarray(rnn.T).astype(bf)
    wlint = np.ascontiguousarray(wlin.T).astype(bf)
    return [
        {
            "rnnb": rnnb,
            "rnnt": rnnt,
            "tgtt": np.ascontiguousarray(tgt[c * TL:(c + 1) * TL].T).astype(bf),
            "wlint": wlint,
            "blin": blin,
            "wsb": wsb,
        }
        for c in range(NCORES)
    ]


def run(inputs, trace=False):
    """Returns (full_output, exec_time_ns_or_None)."""
    _ensure_concourse()
    if trace:
        _wire_ntff_hook()
    from concourse.bass_utils import run_bass_kernel_spmd

    nc = build_program()
    in_maps = make_in_maps(
        inputs["rnn_outputs"], inputs["target"], inputs["W_lin"],
        inputs["b_lin"], inputs["w_score"],
    )
    res = run_bass_kernel_spmd(
        nc, in_maps, core_ids=list(range(NCORES)), trace=trace
    )
    out = np.concatenate(
        [np.asarray(res.results[c]["out"]) for c in range(NCORES)], axis=0
    )
    return out.astype(np.float32), res.exec_time_ns


def kernel(**inputs) -> np.ndarray:
    out, _ = run(inputs, trace=False)
    return out


# revision 10
# speedup vs baseline: 1.0571x; 1.0571x over previous
"""Trainium2 Bass kernel for a Bahdanau-style batch attention layer.

  A = rnn @ W1.T            [S, D]    (W1 = W_lin[:, :DU])
  B = tgt @ W2.T + b_lin    [T, D]    (W2 = W_lin[:, DU:])
  scores[t, s] = w_score . tanh(A[s] + B[t])   (+ b_score, softmax-invariant)
  out = softmax_s(scores) @ rnn                [T, DU]

Sharding: T (target rows) split across 8 NeuronCores; rnn/W replicated.
Host staging pre-transposes/casts the replicated operands so each core's
layout is [d-on-partitions, *-on-free]: the outer-sum A[s]+B[t] is a
per-partition tensor_scalar add on DVE, tanh is a long-free-dim ScalarE
activation (the roofline engine: T*S*D/8 = 33.5M LUT evals per core), and
the score reduction over d is 8 accumulating M=1 matmuls per target row.
"""

import sys
import types

import numpy as np

S = 512
T = 512
DU = 512
DT = 512
D = DU + DT
NCORES = 8
TL = T // NCORES  # 64 target rows per core
TB = 8            # t-block size in the main loop
NTB = TL // TB    # 8 blocks
KD = D // 128     # 8 tiles over d
KS = S // 128     # 4 tiles over s


def _ensure_concourse():
    try:
        import concourse  # noqa: F401
    except ImportError:
        for p in ("/opt/trn_rl_repo", "/root/.axon_site/_ro/trn_rl_repo"):
            if p not in sys.path:
                sys.path.append(p)


def _wire_ntff_hook():
    """Register the NTFF profile hook if the image's antenv lacks it."""
    try:
        import antenv
        if hasattr(antenv, "axon_hooks"):
            return
        mod = types.ModuleType("antenv.axon_hooks")
        mod._hook = None
        def set_axon_ntff_profile_hook(h):
            mod._hook = h
        def get_axon_ntff_profile_hook():
            return mod._hook
        mod.set_axon_ntff_profile_hook = set_axon_ntff_profile_hook
        mod.get_axon_ntff_profile_hook = get_axon_ntff_profile_hook
        sys.modules["antenv.axon_hooks"] = mod
        antenv.axon_hooks = mod
        from trn_agent_boot.trn_boot import _ntff_profile_via_ctypes
        hook = _ntff_profile_via_ctypes("/opt/axon/libaxon_pjrt.so")
        if hook is not None:
            set_axon_ntff_profile_hook(hook)
    except Exception:
        pass


_NC_CACHE = {}


def build_program():
    if "nc" in _NC_CACHE:
        return _NC_CACHE["nc"]
    _ensure_concourse()
    import concourse.bacc as bacc
    import concourse.tile as tile
    from concourse import mybir
    from concourse.masks import make_identity

    f32 = mybir.dt.float32
    bf16 = mybir.dt.bfloat16
    AF = mybir.ActivationFunctionType
    ALU = mybir.AluOpType
    AX = mybir.AxisListType

    nc = bacc.Bacc("TRN2", target_bir_lowering=False, debug=False)

    rnnb_d = nc.dram_tensor("rnnb", [S, DU], bf16, kind="ExternalInput")
    rnnt_d = nc.dram_tensor("rnnt", [DU, S], bf16, kind="ExternalInput")
    tgtt_d = nc.dram_tensor("tgtt", [DT, TL], bf16, kind="ExternalInput")
    wlint_d = nc.dram_tensor("wlint", [D, D], bf16, kind="ExternalInput")
    blin_d = nc.dram_tensor("blin", [128, KD], f32, kind="ExternalInput")
    wsb_d = nc.dram_tensor("wsb", [128, KD], bf16, kind="ExternalInput")
    out_d = nc.dram_tensor("out", [TL, DU], f32, kind="ExternalOutput")

    with tile.TileContext(nc) as tc:
        with (
            tc.tile_pool(name="consts", bufs=1) as consts,
            tc.tile_pool(name="misc", bufs=4) as misc,
            tc.tile_pool(name="hbuf", bufs=2) as hbuf,
            tc.tile_pool(name="ps", bufs=8, space="PSUM") as psp,
        ):
            # ---------------- constants / inputs ----------------
            junk = consts.tile([128, 1], f32)
            nc.gpsimd.memset(junk[:], 0.0)
            # dummy activation: pulls the ~2.7us ACT table load to t=0
            nc.scalar.activation(junk[:], junk[:], AF.Tanh)
            ident_bf = consts.tile([128, 128], bf16)
            make_identity(nc, ident_bf)

            # DMA triggers cost ~0.6us each on the issuing sequencer; spread
            # the prologue loads across the scalar + sync queues and order them
            # by when the per-dj pipeline consumes them.
            rnnT = consts.tile([128, KS, S], bf16)     # [p(k), ki, s]
            for ki in range(KS):
                nc.scalar.dma_start(rnnT[:, ki, :], rnnt_d[ki * 128:(ki + 1) * 128, :])
            tgtT = consts.tile([128, KS, TL], bf16)    # [p(k), ki, t]
            for ki in range(0, KS, 2):
                nc.sync.dma_start(
                    tgtT[:, ki:ki + 2, :], tgtt_d[ki * 128:(ki + 2) * 128, :].rearrange("(a p) t -> p a t", p=128)
                )
            bl_sb = consts.tile([128, KD], f32)
            nc.sync.dma_start(bl_sb[:], blin_d[:])
            ws_bf = consts.tile([128, KD], bf16)
            nc.sync.dma_start(ws_bf[:], wsb_d[:])
            # W^T loaded block-wise, in the order the per-dj prologue consumes it
            wlT = consts.tile([128, KD, KD, 128], bf16)  # [p(k), ki, dj, 128(d)]
            # dj=0 column: 4 blocks from scalar, 4 from sync (land earliest)
            for ki in range(KS):
                nc.scalar.dma_start(
                    wlT[:, ki, 0, :], wlint_d[ki * 128:(ki + 1) * 128, 0:128]
                )
                nc.sync.dma_start(
                    wlT[:, KS + ki, 0, :], wlint_d[(KS + ki) * 128:(KS + ki + 1) * 128, 0:128]
                )
            # dj>=1 in column pairs: 8 triggers per 2 columns keeps the issue
            # rate ahead of the ~2.9us/dj consumption rate
            for dj0 in range(1, KD, 2):
                w = min(2, KD - dj0)
                for ki in range(KD):
                    nc.sync.dma_start(
                        wlT[:, ki, dj0:dj0 + w, :],
                        wlint_d[ki * 128:(ki + 1) * 128, dj0 * 128:(dj0 + w) * 128].rearrange("p (a c) -> p a c", c=128),
                    )

            AT_sb = consts.tile([128, KD, S], bf16)    # [p(d), dj, s]
            BbT_sb = consts.tile([128, KD, TL], f32)   # [p(d), dj, t]
            scores_sb = consts.tile([TL, S], f32)
            h_tiles = [None] * NTB

            def emit_h(tb, dj):
                h = h_tiles[tb]
                for tl in range(TB):
                    t = tb * TB + tl
                    nc.vector.tensor_scalar_add(
                        h[:, dj, tl * S:(tl + 1) * S],
                        AT_sb[:, dj, :],
                        BbT_sb[:, dj, t:t + 1],
                    )
                nc.scalar.activation(h[:, dj, :], h[:, dj, :], AF.Tanh)

            # ---------------- prologue: A^T, Bb^T (+ block-0 h) ----------------
            h_tiles[0] = hbuf.tile([128, KD, TB * S], bf16, tag="h", name="h0")
            for dj in range(KD):
                at_ps = psp.tile([128, 512], f32, tag="ps")
                for ki in range(KS):
                    nc.tensor.matmul(
                        at_ps[:], wlT[:, ki, dj, :], rnnT[:, ki, :],
                        start=(ki == 0), stop=(ki == KS - 1),
                    )
                nc.vector.tensor_copy(AT_sb[:, dj, :], at_ps[:])
                bt_ps = psp.tile([128, 512], f32, tag="ps")
                for ki in range(KS):
                    nc.tensor.matmul(
                        bt_ps[:, :TL], wlT[:, KS + ki, dj, :],
                        tgtT[:, ki, :],
                        start=(ki == 0), stop=(ki == KS - 1),
                    )
                nc.vector.tensor_scalar_add(
                    BbT_sb[:, dj, :], bt_ps[:, :TL], bl_sb[:, dj:dj + 1]
                )
                emit_h(0, dj)
            # tail-only operand: loaded after the prologue-critical DMAs
            rnn_bf = consts.tile([128, KS, DU], bf16)  # [p(s), si, du]
            for si in range(KS):
                nc.gpsimd.dma_start(rnn_bf[:, si, :], rnnb_d[si * 128:(si + 1) * 128, :])

            # ---------------- main loop ----------------
            def emit_block(tb):
                h_tiles[tb] = hbuf.tile([128, KD, TB * S], bf16, tag="h", name=f"h{tb}")
                for dj in range(KD):
                    emit_h(tb, dj)

            def emit_scores(tb):
                h = h_tiles[tb]
                scs = [psp.tile([1, 512], f32, tag="ps", name=f"sc{tb}_{i}") for i in range(TB)]
                for dj in range(KD):
                    for tl in range(TB):
                        nc.tensor.matmul(
                            scs[tl][:], ws_bf[:, dj:dj + 1],
                            h[:, dj, tl * S:(tl + 1) * S],
                            start=(dj == 0), stop=(dj == KD - 1),
                        )
                stage = misc.tile([1, TB * S], f32, tag="stage", name=f"stage{tb}", bufs=2)
                last = tb == NTB - 1
                for tl in range(TB):
                    # last block: split the psum evacuations between DVE and the
                    # (by then idle) scalar engine to shorten the tail
                    if last and tl % 2:
                        nc.scalar.copy(stage[:, tl * S:(tl + 1) * S], scs[tl][:])
                    else:
                        nc.vector.tensor_copy(stage[:, tl * S:(tl + 1) * S], scs[tl][:])
                nc.sync.dma_start(
                    scores_sb[tb * TB:(tb + 1) * TB, :],
                    stage[:],
                )

            for tb in range(1, NTB):
                emit_block(tb)
                emit_scores(tb - 1)
            emit_scores(NTB - 1)

            # ---------------- softmax over s ----------------
            # |scores| <= sum|w_d| (tanh in [-1,1]) is far below exp overflow;
            # skip the max-subtraction (softmax is shift-invariant).  The 1/sum
            # normalization is folded into the final output scale, so the
            # transpose + matmul consume exp() directly.
            e_sb = misc.tile([TL, S], bf16, tag="esb", bufs=1)
            nc.scalar.activation(e_sb[:], scores_sb[:], AF.Exp)
            ssum = misc.tile([TL, 1], f32, tag="ssum", bufs=1)
            nc.vector.tensor_reduce(ssum[:], e_sb[:], axis=AX.X, op=ALU.add)
            rsum = misc.tile([TL, 1], f32, tag="rsum", bufs=1)
            nc.vector.reciprocal(rsum[:], ssum[:])

            # ---------------- out = diag(1/sum) @ (e @ rnn) ----------------
            eT = misc.tile([128, KS, TL], bf16, tag="eT", bufs=1)
            for sj in range(KS):
                tp = psp.tile([128, 512], bf16, tag="ps")
                nc.tensor.transpose(
                    tp[:128, :TL], e_sb[:, sj * 128:(sj + 1) * 128],
                    ident_bf[:TL, :TL],
                )
                nc.vector.tensor_copy(eT[:, sj, :], tp[:, :TL])
            out_ps = psp.tile([TL, DU], f32, tag="ps")
            for ki in range(KS):
                nc.tensor.matmul(
                    out_ps[:], eT[:, ki, :], rnn_bf[:, ki, :],
                    start=(ki == 0), stop=(ki == KS - 1),
                )
            out_sb = misc.tile([TL, DU], f32, tag="osb", bufs=1)
            nc.vector.tensor_scalar_mul(out_sb[:], out_ps[:], rsum[:, 0:1])
            nc.sync.dma_start(out_d[:], out_sb[:])

    nc.compile()
    _NC_CACHE["nc"] = nc
    return nc


def make_in_maps(rnn_outputs, target, W_lin, b_lin, w_score):
    import ml_dtypes
    bf = ml_dtypes.bfloat16
    rnn = np.asarray(rnn_outputs, dtype=np.float32)
    tgt = np.asarray(target, dtype=np.float32)
    wlin = np.asarray(W_lin, dtype=np.float32)
    blin = np.asarray(b_lin, dtype=np.float32).reshape(KD, 128).T.copy()
    wsb = np.asarray(w_score, dtype=np.float32).reshape(KD, 128).T.astype(bf)
    rnnb = rnn.astype(bf)
    rnnt = np.ascontiguousarray(rnn.T).astype(bf)
    wlint = np.ascontiguousarray(wlin.T).astype(bf)
    return [
        {
            "rnnb": rnnb,
            "rnnt": rnnt,
            "tgtt": np.ascontiguousarray(tgt[c * TL:(c + 1) * TL].T).astype(bf),
            "wlint": wlint,
            "blin": blin,
            "wsb": wsb,
        }
        for c in range(NCORES)
    ]


def run(inputs, trace=False):
    """Returns (full_output, exec_time_ns_or_None)."""
    _ensure_concourse()
    if trace:
        _wire_ntff_hook()
    from concourse.bass_utils import run_bass_kernel_spmd

    nc = build_program()
    in_maps = make_in_maps(
        inputs["rnn_outputs"], inputs["target"], inputs["W_lin"],
        inputs["b_lin"], inputs["w_score"],
    )
    res = run_bass_kernel_spmd(
        nc, in_maps, core_ids=list(range(NCORES)), trace=trace
    )
    out = np.concatenate(
        [np.asarray(res.results[c]["out"]) for c in range(NCORES)], axis=0
    )
    return out.astype(np.float32), res.exec_time_ns


def kernel(**inputs) -> np.ndarray:
    out, _ = run(inputs, trace=False)
    return out


# revision 11
# speedup vs baseline: 1.0638x; 1.0063x over previous
"""Trainium2 Bass kernel for a Bahdanau-style batch attention layer.

  A = rnn @ W1.T            [S, D]    (W1 = W_lin[:, :DU])
  B = tgt @ W2.T + b_lin    [T, D]    (W2 = W_lin[:, DU:])
  scores[t, s] = w_score . tanh(A[s] + B[t])   (+ b_score, softmax-invariant)
  out = softmax_s(scores) @ rnn                [T, DU]

Sharding: T (target rows) split across 8 NeuronCores; rnn/W replicated.
Host staging pre-transposes/casts the replicated operands so each core's
layout is [d-on-partitions, *-on-free]: the outer-sum A[s]+B[t] is a
per-partition tensor_scalar add on DVE, tanh is a long-free-dim ScalarE
activation (the roofline engine: T*S*D/8 = 33.5M LUT evals per core), and
the score reduction over d is 8 accumulating M=1 matmuls per target row.
"""

import sys
import types

import numpy as np

S = 512
T = 512
DU = 512
DT = 512
D = DU + DT
NCORES = 8
TL = T // NCORES  # 64 target rows per core
TB = 8            # t-block size in the main loop
NTB = TL // TB    # 8 blocks
KD = D // 128     # 8 tiles over d
KS = S // 128     # 4 tiles over s


def _ensure_concourse():
    try:
        import concourse  # noqa: F401
    except ImportError:
        for p in ("/opt/trn_rl_repo", "/root/.axon_site/_ro/trn_rl_repo"):
            if p not in sys.path:
                sys.path.append(p)


def _wire_ntff_hook():
    """Register the NTFF profile hook if the image's antenv lacks it."""
    try:
        import antenv
        if hasattr(antenv, "axon_hooks"):
            return
        mod = types.ModuleType("antenv.axon_hooks")
        mod._hook = None
        def set_axon_ntff_profile_hook(h):
            mod._hook = h
        def get_axon_ntff_profile_hook():
            return mod._hook
        mod.set_axon_ntff_profile_hook = set_axon_ntff_profile_hook
        mod.get_axon_ntff_profile_hook = get_axon_ntff_profile_hook
        sys.modules["antenv.axon_hooks"] = mod
        antenv.axon_hooks = mod
        from trn_agent_boot.trn_boot import _ntff_profile_via_ctypes
        hook = _ntff_profile_via_ctypes("/opt/axon/libaxon_pjrt.so")
        if hook is not None:
            set_axon_ntff_profile_hook(hook)
    except Exception:
        pass


_NC_CACHE = {}


def build_program():
    if "nc" in _NC_CACHE:
        return _NC_CACHE["nc"]
    _ensure_concourse()
    import concourse.bacc as bacc
    import concourse.tile as tile
    from concourse import mybir
    from concourse.masks import make_identity

    f32 = mybir.dt.float32
    bf16 = mybir.dt.bfloat16
    AF = mybir.ActivationFunctionType
    ALU = mybir.AluOpType
    AX = mybir.AxisListType

    nc = bacc.Bacc("TRN2", target_bir_lowering=False, debug=False)

    rnnb_d = nc.dram_tensor("rnnb", [S, DU], bf16, kind="ExternalInput")
    rnnt_d = nc.dram_tensor("rnnt", [DU, S], bf16, kind="ExternalInput")
    tgtt_d = nc.dram_tensor("tgtt", [DT, TL], bf16, kind="ExternalInput")
    wlint_d = nc.dram_tensor("wlint", [D, D], bf16, kind="ExternalInput")
    blin_d = nc.dram_tensor("blin", [128, KD], f32, kind="ExternalInput")
    wsb_d = nc.dram_tensor("wsb", [128, KD], bf16, kind="ExternalInput")
    out_d = nc.dram_tensor("out", [TL, DU], f32, kind="ExternalOutput")

    with tile.TileContext(nc) as tc:
        with (
            tc.tile_pool(name="consts", bufs=1) as consts,
            tc.tile_pool(name="misc", bufs=4) as misc,
            tc.tile_pool(name="hbuf", bufs=2) as hbuf,
            tc.tile_pool(name="ps", bufs=8, space="PSUM") as psp,
        ):
            # ---------------- constants / inputs ----------------
            junk = consts.tile([128, 1], f32)
            nc.gpsimd.memset(junk[:], 0.0)
            ident_bf = consts.tile([128, 128], bf16)
            make_identity(nc, ident_bf)

            # DMA triggers cost ~0.6us each on the issuing sequencer; spread
            # the prologue loads across the scalar + sync queues and order them
            # by when the per-dj pipeline consumes them.
            rnnT = consts.tile([128, KS, S], bf16)     # [p(k), ki, s]
            for ki in range(KS):
                eng = nc.scalar if ki % 2 == 0 else nc.sync
                eng.dma_start(rnnT[:, ki, :], rnnt_d[ki * 128:(ki + 1) * 128, :])
            tgtT = consts.tile([128, KS, TL], bf16)    # [p(k), ki, t]
            for ki in range(0, KS, 2):
                nc.sync.dma_start(
                    tgtT[:, ki:ki + 2, :], tgtt_d[ki * 128:(ki + 2) * 128, :].rearrange("(a p) t -> p a t", p=128)
                )
            bl_sb = consts.tile([128, KD], f32)
            nc.sync.dma_start(bl_sb[:], blin_d[:])
            ws_bf = consts.tile([128, KD], bf16)
            nc.sync.dma_start(ws_bf[:], wsb_d[:])
            # W^T loaded block-wise, in the order the per-dj prologue consumes it
            wlT = consts.tile([128, KD, KD, 128], bf16)  # [p(k), ki, dj, 128(d)]
            # dj=0 column: 4 blocks from scalar, 4 from sync (land earliest)
            for ki in range(KS):
                nc.scalar.dma_start(
                    wlT[:, ki, 0, :], wlint_d[ki * 128:(ki + 1) * 128, 0:128]
                )
                nc.sync.dma_start(
                    wlT[:, KS + ki, 0, :], wlint_d[(KS + ki) * 128:(KS + ki + 1) * 128, 0:128]
                )
            # table load (~2.7us) only gates the first tanh, not the DMAs/adds:
            # issue it on the scalar queue after the critical triggers
            nc.scalar.activation(junk[:], junk[:], AF.Tanh)
            # dj>=1 in column pairs: 8 triggers per 2 columns keeps the issue
            # rate ahead of the ~2.9us/dj consumption rate
            for dj0 in range(1, KD, 2):
                w = min(2, KD - dj0)
                for ki in range(KD):
                    nc.sync.dma_start(
                        wlT[:, ki, dj0:dj0 + w, :],
                        wlint_d[ki * 128:(ki + 1) * 128, dj0 * 128:(dj0 + w) * 128].rearrange("p (a c) -> p a c", c=128),
                    )

            AT_sb = consts.tile([128, KD, S], bf16)    # [p(d), dj, s]
            BbT_sb = consts.tile([128, KD, TL], f32)   # [p(d), dj, t]
            scores_sb = consts.tile([TL, S], f32)
            h_tiles = [None] * NTB

            def emit_adds(tb, dj):
                h = h_tiles[tb]
                for tl in range(TB):
                    t = tb * TB + tl
                    nc.vector.tensor_scalar_add(
                        h[:, dj, tl * S:(tl + 1) * S],
                        AT_sb[:, dj, :],
                        BbT_sb[:, dj, t:t + 1],
                    )

            def emit_h(tb, dj):
                # first/last block: per-dj tanh (fine-grained head/tail
                # streaming); middle blocks: dj-pair tanh (FD 8192 halves the
                # per-instruction ScalarE overhead)
                h = h_tiles[tb]
                if tb in (0, NTB - 1):
                    emit_adds(tb, dj)
                    nc.scalar.activation(h[:, dj, :], h[:, dj, :], AF.Tanh)
                elif dj % 2 == 0:
                    emit_adds(tb, dj)
                else:
                    emit_adds(tb, dj)
                    nc.scalar.activation(
                        h[:, dj - 1:dj + 1, :], h[:, dj - 1:dj + 1, :], AF.Tanh
                    )

            # ---------------- prologue: A^T, Bb^T (+ block-0 h) ----------------
            h_tiles[0] = hbuf.tile([128, KD, TB * S], bf16, tag="h", name="h0")
            for dj in range(KD):
                at_ps = psp.tile([128, 512], f32, tag="ps")
                for ki in range(KS):
                    nc.tensor.matmul(
                        at_ps[:], wlT[:, ki, dj, :], rnnT[:, ki, :],
                        start=(ki == 0), stop=(ki == KS - 1),
                    )
                nc.vector.tensor_copy(AT_sb[:, dj, :], at_ps[:])
                bt_ps = psp.tile([128, 512], f32, tag="ps")
                for ki in range(KS):
                    nc.tensor.matmul(
                        bt_ps[:, :TL], wlT[:, KS + ki, dj, :],
                        tgtT[:, ki, :],
                        start=(ki == 0), stop=(ki == KS - 1),
                    )
                nc.vector.tensor_scalar_add(
                    BbT_sb[:, dj, :], bt_ps[:, :TL], bl_sb[:, dj:dj + 1]
                )
                emit_h(0, dj)
            # tail-only operand: loaded after the prologue-critical DMAs
            rnn_bf = consts.tile([128, KS, DU], bf16)  # [p(s), si, du]
            for si in range(KS):
                nc.gpsimd.dma_start(rnn_bf[:, si, :], rnnb_d[si * 128:(si + 1) * 128, :])

            # ---------------- main loop ----------------
            def emit_block(tb):
                h_tiles[tb] = hbuf.tile([128, KD, TB * S], bf16, tag="h", name=f"h{tb}")
                for dj in range(KD):
                    emit_h(tb, dj)

            def emit_scores(tb):
                h = h_tiles[tb]
                scs = [psp.tile([1, 512], f32, tag="ps", name=f"sc{tb}_{i}") for i in range(TB)]
                for dj in range(KD):
                    for tl in range(TB):
                        nc.tensor.matmul(
                            scs[tl][:], ws_bf[:, dj:dj + 1],
                            h[:, dj, tl * S:(tl + 1) * S],
                            start=(dj == 0), stop=(dj == KD - 1),
                        )
                stage = misc.tile([1, TB * S], f32, tag="stage", name=f"stage{tb}", bufs=2)
                last = tb == NTB - 1
                for tl in range(TB):
                    # last block: split the psum evacuations between DVE and the
                    # (by then idle) scalar engine to shorten the tail
                    if last and tl % 2:
                        nc.scalar.copy(stage[:, tl * S:(tl + 1) * S], scs[tl][:])
                    else:
                        nc.vector.tensor_copy(stage[:, tl * S:(tl + 1) * S], scs[tl][:])
                nc.sync.dma_start(
                    scores_sb[tb * TB:(tb + 1) * TB, :],
                    stage[:],
                )

            for tb in range(1, NTB):
                emit_block(tb)
                emit_scores(tb - 1)
            emit_scores(NTB - 1)

            # ---------------- softmax over s ----------------
            # |scores| <= sum|w_d| (tanh in [-1,1]) is far below exp overflow;
            # skip the max-subtraction (softmax is shift-invariant).  The 1/sum
            # normalization is folded into the final output scale, so the
            # transpose + matmul consume exp() directly.
            e_sb = misc.tile([TL, S], bf16, tag="esb", bufs=1)
            nc.scalar.activation(e_sb[:], scores_sb[:], AF.Exp)
            ssum = misc.tile([TL, 1], f32, tag="ssum", bufs=1)
            nc.vector.tensor_reduce(ssum[:], e_sb[:], axis=AX.X, op=ALU.add)
            rsum = misc.tile([TL, 1], f32, tag="rsum", bufs=1)
            nc.vector.reciprocal(rsum[:], ssum[:])

            # ---------------- out = diag(1/sum) @ (e @ rnn) ----------------
            eT = misc.tile([128, KS, TL], bf16, tag="eT", bufs=1)
            for sj in range(KS):
                tp = psp.tile([128, 512], bf16, tag="ps")
                nc.tensor.transpose(
                    tp[:128, :TL], e_sb[:, sj * 128:(sj + 1) * 128],
                    ident_bf[:TL, :TL],
                )
                nc.vector.tensor_copy(eT[:, sj, :], tp[:, :TL])
            out_ps = psp.tile([TL, DU], f32, tag="ps")
            for ki in range(KS):
                nc.tensor.matmul(
                    out_ps[:], eT[:, ki, :], rnn_bf[:, ki, :],
                    start=(ki == 0), stop=(ki == KS - 1),
                )
            out_sb = misc.tile([TL, DU], f32, tag="osb", bufs=1)
            nc.vector.tensor_scalar_mul(out_sb[:], out_ps[:], rsum[:, 0:1])
            nc.sync.dma_start(out_d[:], out_sb[:])

    nc.compile()
    _NC_CACHE["nc"] = nc
    return nc


def make_in_maps(rnn_outputs, target, W_lin, b_lin, w_score):
    import ml_dtypes
    bf = ml_dtypes.bfloat16
    rnn = np.asarray(rnn_outputs, dtype=np.float32)
    tgt = np.asarray(target, dtype=np.float32)
    wlin = np.asarray(W_lin, dtype=np.float32)
    blin = np.asarray(b_lin, dtype=np.float32).reshape(KD, 128).T.copy()
    wsb = np.asarray(w_score, dtype=np.float32).reshape(KD, 128).T.astype(bf)
    rnnb = rnn.astype(bf)
    rnnt = np.ascontiguousarray(rnn.T).astype(bf)
    wlint = np.ascontiguousarray(wlin.T).astype(bf)
    return [
        {
            "rnnb": rnnb,
            "rnnt": rnnt,
            "tgtt": np.ascontiguousarray(tgt[c * TL:(c + 1) * TL].T).astype(bf),
            "wlint": wlint,
            "blin": blin,
            "wsb": wsb,
        }
        for c in range(NCORES)
    ]


def run(inputs, trace=False):
    """Returns (full_output, exec_time_ns_or_None)."""
    _ensure_concourse()
    if trace:
        _wire_ntff_hook()
    from concourse.bass_utils import run_bass_kernel_spmd

    nc = build_program()
    in_maps = make_in_maps(
        inputs["rnn_outputs"], inputs["target"], inputs["W_lin"],
        inputs["b_lin"], inputs["w_score"],
    )
    res = run_bass_kernel_spmd(
        nc, in_maps, core_ids=list(range(NCORES)), trace=trace
    )
    out = np.concatenate(
        [np.asarray(res.results[c]["out"]) for c in range(NCORES)], axis=0
    )
    return out.astype(np.float32), res.exec_time_ns


def kernel(**inputs) -> np.ndarray:
    out, _ = run(inputs, trace=False)
    return out


# revision 13
# speedup vs baseline: 1.0795x; 1.0148x over previous
"""Trainium2 Bass kernel for a Bahdanau-style batch attention layer.

  A = rnn @ W1.T            [S, D]    (W1 = W_lin[:, :DU])
  B = tgt @ W2.T + b_lin    [T, D]    (W2 = W_lin[:, DU:])
  scores[t, s] = w_score . tanh(A[s] + B[t])   (+ b_score, softmax-invariant)
  out = softmax_s(scores) @ rnn                [T, DU]

Sharding: T (target rows) split across 8 NeuronCores; rnn/W replicated.
Host staging pre-transposes/casts the replicated operands so each core's
layout is [d-on-partitions, *-on-free]: the outer-sum A[s]+B[t] is a
per-partition tensor_scalar add on DVE, tanh is a long-free-dim ScalarE
activation (the roofline engine: T*S*D/8 = 33.5M LUT evals per core), and
the score reduction over d is 8 accumulating M=1 matmuls per target row.
"""

import sys
import types

import numpy as np

S = 512
T = 512
DU = 512
DT = 512
D = DU + DT
NCORES = 8
TL = T // NCORES  # 64 target rows per core
TB = 8            # t-block size in the main loop
NTB = TL // TB    # 8 blocks
KD = D // 128     # 8 tiles over d
KS = S // 128     # 4 tiles over s


def _ensure_concourse():
    try:
        import concourse  # noqa: F401
    except ImportError:
        for p in ("/opt/trn_rl_repo", "/root/.axon_site/_ro/trn_rl_repo"):
            if p not in sys.path:
                sys.path.append(p)


def _wire_ntff_hook():
    """Register the NTFF profile hook if the image's antenv lacks it."""
    try:
        import antenv
        if hasattr(antenv, "axon_hooks"):
            return
        mod = types.ModuleType("antenv.axon_hooks")
        mod._hook = None
        def set_axon_ntff_profile_hook(h):
            mod._hook = h
        def get_axon_ntff_profile_hook():
            return mod._hook
        mod.set_axon_ntff_profile_hook = set_axon_ntff_profile_hook
        mod.get_axon_ntff_profile_hook = get_axon_ntff_profile_hook
        sys.modules["antenv.axon_hooks"] = mod
        antenv.axon_hooks = mod
        from trn_agent_boot.trn_boot import _ntff_profile_via_ctypes
        hook = _ntff_profile_via_ctypes("/opt/axon/libaxon_pjrt.so")
        if hook is not None:
            set_axon_ntff_profile_hook(hook)
    except Exception:
        pass


_NC_CACHE = {}


def build_program():
    if "nc" in _NC_CACHE:
        return _NC_CACHE["nc"]
    _ensure_concourse()
    import concourse.bacc as bacc
    import concourse.tile as tile
    from concourse import mybir
    from concourse.masks import make_identity

    f32 = mybir.dt.float32
    bf16 = mybir.dt.bfloat16
    AF = mybir.ActivationFunctionType
    ALU = mybir.AluOpType
    AX = mybir.AxisListType

    nc = bacc.Bacc("TRN2", target_bir_lowering=False, debug=False)

    rnnb_d = nc.dram_tensor("rnnb", [S, DU], bf16, kind="ExternalInput")
    rnnt_d = nc.dram_tensor("rnnt", [DU, S], bf16, kind="ExternalInput")
    tgtt_d = nc.dram_tensor("tgtt", [DT, TL], bf16, kind="ExternalInput")
    wlint_d = nc.dram_tensor("wlint", [D, D], bf16, kind="ExternalInput")
    small_d = nc.dram_tensor("small", [128, 2 * KD], f32, kind="ExternalInput")
    out_d = nc.dram_tensor("out", [TL, DU], f32, kind="ExternalOutput")

    with tile.TileContext(nc) as tc:
        with (
            tc.tile_pool(name="consts", bufs=1) as consts,
            tc.tile_pool(name="misc", bufs=4) as misc,
            tc.tile_pool(name="hbuf", bufs=2) as hbuf,
            tc.tile_pool(name="ps", bufs=8, space="PSUM") as psp,
        ):
            # ---------------- constants / inputs ----------------
            junk = consts.tile([128, 1], f32)
            nc.gpsimd.memset(junk[:], 0.0)
            ident_bf = consts.tile([128, 128], bf16)
            make_identity(nc, ident_bf)

            # DMA triggers cost ~0.6us each on the issuing sequencer; spread
            # the prologue loads across the scalar + sync queues and order them
            # by when the per-dj pipeline consumes them.
            # scalar issues the dj0-critical loads (8 triggers), then the ACT
            # table load; sync issues everything else, dj-pair-batched in
            # consumption order.  Trigger cost is ~0.6us per [128,*] DMA.
            rnnT = consts.tile([128, KS, S], bf16)     # [p(k), ki, s]
            wlT = consts.tile([128, KD, KD, 128], bf16)  # [p(k), ki, dj, 128(d)]
            for ki in range(KS):
                nc.scalar.dma_start(rnnT[:, ki, :], rnnt_d[ki * 128:(ki + 1) * 128, :])
            for ki in range(KS):
                nc.scalar.dma_start(
                    wlT[:, ki, 0, :], wlint_d[ki * 128:(ki + 1) * 128, 0:128]
                )
            # table load (~2.7us) only gates the first tanh, not the DMAs/adds
            nc.scalar.activation(junk[:], junk[:], AF.Tanh)

            for ki in range(KS):
                nc.sync.dma_start(
                    wlT[:, KS + ki, 0, :], wlint_d[(KS + ki) * 128:(KS + ki + 1) * 128, 0:128]
                )
            tgtT = consts.tile([128, KS, TL], bf16)    # [p(k), ki, t]
            for ki in range(0, KS, 2):
                nc.sync.dma_start(
                    tgtT[:, ki:ki + 2, :], tgtt_d[ki * 128:(ki + 2) * 128, :].rearrange("(a p) t -> p a t", p=128)
                )
            small_sb = consts.tile([128, 2 * KD], f32)
            nc.sync.dma_start(small_sb[:], small_d[:])
            bl_sb = small_sb[:, 0:KD]
            ws_bf = consts.tile([128, KD], bf16)
            nc.vector.tensor_copy(ws_bf[:], small_sb[:, KD:2 * KD])
            # dj>=1 in column pairs: 8 triggers per 2 columns keeps the issue
            # rate ahead of the ~2.9us/dj consumption rate
            for dj0 in range(1, KD, 2):
                w = min(2, KD - dj0)
                for ki in range(KD):
                    nc.sync.dma_start(
                        wlT[:, ki, dj0:dj0 + w, :],
                        wlint_d[ki * 128:(ki + 1) * 128, dj0 * 128:(dj0 + w) * 128].rearrange("p (a c) -> p a c", c=128),
                    )

            AT_sb = consts.tile([128, KD, S], bf16)    # [p(d), dj, s]
            BbT_sb = consts.tile([128, KD, TL], f32)   # [p(d), dj, t]
            scores_sb = consts.tile([TL, S], f32)
            h_tiles = [None] * NTB

            def emit_adds(tb, dj):
                h = h_tiles[tb]
                for tl in range(TB):
                    t = tb * TB + tl
                    nc.vector.tensor_scalar_add(
                        h[:, dj, tl * S:(tl + 1) * S],
                        AT_sb[:, dj, :],
                        BbT_sb[:, dj, t:t + 1],
                    )

            def emit_h(tb, dj):
                # first/last block: per-dj tanh (fine-grained head/tail
                # streaming); middle blocks: dj-pair tanh (FD 8192 halves the
                # per-instruction ScalarE overhead)
                h = h_tiles[tb]
                if tb in (0, NTB - 1):
                    emit_adds(tb, dj)
                    nc.scalar.activation(h[:, dj, :], h[:, dj, :], AF.Tanh)
                elif dj % 2 == 0:
                    emit_adds(tb, dj)
                else:
                    emit_adds(tb, dj)
                    nc.scalar.activation(
                        h[:, dj - 1:dj + 1, :], h[:, dj - 1:dj + 1, :], AF.Tanh
                    )

            # ---------------- prologue: A^T, Bb^T (+ block-0 h) ----------------
            h_tiles[0] = hbuf.tile([128, KD, TB * S], bf16, tag="h", name="h0")
            for dj in range(KD):
                at_ps = psp.tile([128, 512], f32, tag="ps")
                for ki in range(KS):
                    nc.tensor.matmul(
                        at_ps[:], wlT[:, ki, dj, :], rnnT[:, ki, :],
                        start=(ki == 0), stop=(ki == KS - 1),
                    )
                nc.vector.tensor_copy(AT_sb[:, dj, :], at_ps[:])
                bt_ps = psp.tile([128, 512], f32, tag="ps")
                for ki in range(KS):
                    nc.tensor.matmul(
                        bt_ps[:, :TL], wlT[:, KS + ki, dj, :],
                        tgtT[:, ki, :],
                        start=(ki == 0), stop=(ki == KS - 1),
                    )
                nc.vector.tensor_scalar_add(
                    BbT_sb[:, dj, :], bt_ps[:, :TL], bl_sb[:, dj:dj + 1]
                )
                emit_h(0, dj)
            # tail-only operand: loaded after the prologue-critical DMAs
            rnn_bf = consts.tile([128, KS, DU], bf16)  # [p(s), si, du]
            for si in range(KS):
                nc.gpsimd.dma_start(rnn_bf[:, si, :], rnnb_d[si * 128:(si + 1) * 128, :])

            # ---------------- main loop ----------------
            def emit_block(tb):
                h_tiles[tb] = hbuf.tile([128, KD, TB * S], bf16, tag="h", name=f"h{tb}")
                for dj in range(KD):
                    emit_h(tb, dj)

            def emit_scores(tb):
                h = h_tiles[tb]
                scs = [psp.tile([1, 512], f32, tag="ps", name=f"sc{tb}_{i}") for i in range(TB)]
                for dj in range(KD):
                    for tl in range(TB):
                        nc.tensor.matmul(
                            scs[tl][:], ws_bf[:, dj:dj + 1],
                            h[:, dj, tl * S:(tl + 1) * S],
                            start=(dj == 0), stop=(dj == KD - 1),
                        )
                stage = misc.tile([1, TB * S], f32, tag="stage", name=f"stage{tb}", bufs=2)
                last = tb == NTB - 1
                for tl in range(TB):
                    # last block: split the psum evacuations between DVE and the
                    # (by then idle) scalar engine to shorten the tail
                    if last and tl % 2:
                        nc.scalar.copy(stage[:, tl * S:(tl + 1) * S], scs[tl][:])
                    else:
                        nc.vector.tensor_copy(stage[:, tl * S:(tl + 1) * S], scs[tl][:])
                nc.sync.dma_start(
                    scores_sb[tb * TB:(tb + 1) * TB, :],
                    stage[:],
                )

            for tb in range(1, NTB):
                emit_block(tb)
                emit_scores(tb - 1)
            emit_scores(NTB - 1)

            # ---------------- softmax over s ----------------
            # |scores| <= sum|w_d| (tanh in [-1,1]) is far below exp overflow;
            # skip the max-subtraction (softmax is shift-invariant).  The 1/sum
            # normalization is folded into the final output scale, so the
            # transpose + matmul consume exp() directly.
            e_sb = misc.tile([TL, S], bf16, tag="esb", bufs=1)
            nc.scalar.activation(e_sb[:], scores_sb[:], AF.Exp)
            ssum = misc.tile([TL, 1], f32, tag="ssum", bufs=1)
            nc.vector.tensor_reduce(ssum[:], e_sb[:], axis=AX.X, op=ALU.add)
            rsum = misc.tile([TL, 1], f32, tag="rsum", bufs=1)
            nc.vector.reciprocal(rsum[:], ssum[:])

            # ---------------- out = diag(1/sum) @ (e @ rnn) ----------------
            eT = misc.tile([128, KS, TL], bf16, tag="eT", bufs=1)
            for sj in range(KS):
                tp = psp.tile([128, 512], bf16, tag="ps")
                nc.tensor.transpose(
                    tp[:128, :TL], e_sb[:, sj * 128:(sj + 1) * 128],
                    ident_bf[:TL, :TL],
                )
                nc.vector.tensor_copy(eT[:, sj, :], tp[:, :TL])
            out_ps = psp.tile([TL, DU], f32, tag="ps")
            for ki in range(KS):
                nc.tensor.matmul(
                    out_ps[:], eT[:, ki, :], rnn_bf[:, ki, :],
                    start=(ki == 0), stop=(ki == KS - 1),
                )
            out_sb = misc.tile([TL, DU], f32, tag="osb", bufs=1)
            nc.vector.tensor_scalar_mul(out_sb[:], out_ps[:], rsum[:, 0:1])
            nc.sync.dma_start(out_d[:], out_sb[:])

    nc.compile()
    _NC_CACHE["nc"] = nc
    return nc


def make_in_maps(rnn_outputs, target, W_lin, b_lin, w_score):
    import ml_dtypes
    bf = ml_dtypes.bfloat16
    rnn = np.asarray(rnn_outputs, dtype=np.float32)
    tgt = np.asarray(target, dtype=np.float32)
    wlin = np.asarray(W_lin, dtype=np.float32)
    blin = np.asarray(b_lin, dtype=np.float32).reshape(KD, 128).T
    wsb = np.asarray(w_score, dtype=np.float32).reshape(KD, 128).T
    small = np.ascontiguousarray(np.concatenate([blin, wsb], axis=1))
    rnnb = rnn.astype(bf)
    rnnt = np.ascontiguousarray(rnn.T).astype(bf)
    wlint = np.ascontiguousarray(wlin.T).astype(bf)
    return [
        {
            "rnnb": rnnb,
            "rnnt": rnnt,
            "tgtt": np.ascontiguousarray(tgt[c * TL:(c + 1) * TL].T).astype(bf),
            "wlint": wlint,
            "small": small,
        }
        for c in range(NCORES)
    ]


def run(inputs, trace=False):
    """Returns (full_output, exec_time_ns_or_None)."""
    _ensure_concourse()
    if trace:
        _wire_ntff_hook()
    from concourse.bass_utils import run_bass_kernel_spmd

    nc = build_program()
    in_maps = make_in_maps(
        inputs["rnn_outputs"], inputs["target"], inputs["W_lin"],
        inputs["b_lin"], inputs["w_score"],
    )
    res = run_bass_kernel_spmd(
        nc, in_maps, core_ids=list(range(NCORES)), trace=trace
    )
    out = np.concatenate(
        [np.asarray(res.results[c]["out"]) for c in range(NCORES)], axis=0
    )
    return out.astype(np.float32), res.exec_time_ns


def kernel(**inputs) -> np.ndarray:
    out, _ = run(inputs, trace=False)
    return out
